# revision 17
# baseline (speedup 1.0000x reference)
"""Trainium2 Bass kernel for the LogicLayer (gnn_message_passing) problem.

out[n, y] = k0[y] + k1[y]*a + k2[y]*b + k3[y]*(a*b)
  with a = x[n, a_idx[y]], b = x[n, b_idx[y]],
  k = softmax(weights, -1) @ GATE_COEFFS          (per output neuron y)

Strategy (8 NeuronCores, sharded over out_dim — 2048 neurons/core, full
batch). The kernel is HBM-bandwidth bound, so all bulk traffic is 16-bit:
  * x is uploaded transposed and recentered in fp16: xT = fp16(x - 0.5),
    [16384, 4096] feature-major. The 0.5 shift halves the fp16
    quantization error (x' in [-0.5, 0.5)) and is folded into the gate
    coefficient matrix on the host (out = C0 + C1 a' + C2 b' + C3 a'b').
  * Per-core on-device softmax of the core's weight slice gives coefficient
    tiles kg[j][q, t] = C_j(y = t*128 + q) directly.
  * Per 128-output chunk t: ONE dma_gather with 256 indices (the chunk's
    a-rows then b-rows, 8KB/descriptor, 2MB/op) lands AB [128, 2, 4096]
    in SBUF. u = C1*A + C0 is split between DVE tensor_scalar (odd-length
    slice -> 2x_1P single-port mode) and ACT; ACT computes w = C3*A + C2
    (free affine); DVE computes w *= B and osb = u + w (fp16 tensor_tensor,
    2x mode); one contiguous 2MB DMA stores each pair of output row blocks.
  * Output is produced neuron-major ([out, batch] fp16); the host
    transposes/casts back to [batch, out] f32. Total HBM traffic per core:
    32MB gather + 16MB store (vs 64+32 for the f32 batch-major variant).
"""

import numpy as np

_GATE_COEFFS = np.array(
    [
        [0.0, 0.0, 0.0, 0.0],
        [0.0, 0.0, 0.0, 1.0],
        [0.0, 1.0, 0.0, -1.0],
        [0.0, 1.0, 0.0, 0.0],
        [0.0, 0.0, 1.0, -1.0],
        [0.0, 0.0, 1.0, 0.0],
        [0.0, 1.0, 1.0, -2.0],
        [0.0, 1.0, 1.0, -1.0],
        [1.0, -1.0, -1.0, 1.0],
        [1.0, -1.0, -1.0, 2.0],
        [1.0, 0.0, -1.0, 0.0],
        [1.0, 0.0, -1.0, 1.0],
        [1.0, -1.0, 0.0, 0.0],
        [1.0, -1.0, 0.0, 1.0],
        [1.0, 0.0, 0.0, -1.0],
        [1.0, 0.0, 0.0, 0.0],
    ],
    dtype=np.float32,
)

# x is stored recentered (x' = x - H); fold the shift into the coefficients:
# out = k0 + k1(a'+H) + k2(b'+H) + k3(a'+H)(b'+H) = C0 + C1 a' + C2 b' + C3 a'b'
_H = 0.5
_SHIFT_T = np.array(
    [
        [1.0, _H, _H, _H * _H],
        [0.0, 1.0, 0.0, _H],
        [0.0, 0.0, 1.0, _H],
        [0.0, 0.0, 0.0, 1.0],
    ],
    dtype=np.float32,
)
_GATE_COEFFS_C = _GATE_COEFFS @ _SHIFT_T.T  # [16, 4]: softmax(w) @ this = C

BATCH, IN_DIM, OUT_DIM = 4096, 16384, 16384
NCORES = 8
OC = OUT_DIM // NCORES   # 2048 outputs per core
NCHUNK = OC // 128       # 16 chunks of 128 outputs
IDX_PER_OP = 256         # one chunk's a+b rows in one dma_gather
NGOP = OC * 2 // IDX_PER_OP  # 16 gather ops per core

_PROGRAM_CACHE = {}


def _wrap_idx(a_slice: np.ndarray, b_slice: np.ndarray) -> np.ndarray:
    """dma_gather wrapped-int16 layout. Gather op o fetches 512 rows
    (a[2o]*128, b[2o]*128, a[2o+1]*128, b[2o+1]*128 — idx i lands in
    partition i%128, slot i//128); item i of op o lives at
    [i % 16, o*32 + i//16], replicated across the 8 16-partition groups."""
    idx = np.stack([a_slice.reshape(NCHUNK, 128), b_slice.reshape(NCHUNK, 128)], 1)
    idx = idx.reshape(NGOP, IDX_PER_OP).astype(np.int16)        # [o, i]
    w = idx.reshape(NGOP, IDX_PER_OP // 16, 16)                 # [o, s, p16]
    w = np.ascontiguousarray(w.transpose(2, 0, 1)).reshape(16, NGOP * 16)
    return np.ascontiguousarray(np.tile(w, (8, 1)))


def _build_program():
    import concourse.bass as bass  # noqa: F401
    import concourse.tile as tile
    from concourse import bacc, mybir

    f32 = mybir.dt.float32
    f16 = mybir.dt.float16
    i16 = mybir.dt.int16
    AF = mybir.ActivationFunctionType
    ALU = mybir.AluOpType

    nc = bacc.Bacc("TRN2", target_bir_lowering=False, debug=False)
    xT_h = nc.dram_tensor("xT", [IN_DIM, BATCH], f16, kind="ExternalInput")
    w_h = nc.dram_tensor("w16c", [OC, 16], f32, kind="ExternalInput")
    iab_h = nc.dram_tensor("iab", [128, NGOP * 16], i16, kind="ExternalInput")
    gm_h = nc.dram_tensor("gm", [4, 128, 256], f32, kind="ExternalInput")
    out_h = nc.dram_tensor("out", [OC, BATCH], f16, kind="ExternalOutput")

    with tile.TileContext(nc) as tc:
        from contextlib import ExitStack

        with ExitStack() as stack:
            cp = stack.enter_context(tc.tile_pool(name="const", bufs=1))
            # all pools coexist: no SBUF reuse between the coefficient calc
            # and the main loop, so the first gathers start immediately and
            # overlap the (serial, small) softmax chain.
            kp = stack.enter_context(tc.tile_pool(name="kcalc", bufs=1))
            pab = stack.enter_context(tc.tile_pool(name="pab", bufs=7))
            puw = stack.enter_context(tc.tile_pool(name="puw", bufs=2))
            po = stack.enter_context(tc.tile_pool(name="po", bufs=2))

            iab_sb = cp.tile([128, NGOP * 16], i16)
            nc.sync.dma_start(iab_sb[:], iab_h.ap()[:, :])
            kg = [
                cp.tile([128, NCHUNK], f32, tag=f"kg{j}", name=f"kg{j}")
                for j in range(4)
            ]

            # ---- coefficients: C = softmax(weights_slice) @ GATE_COEFFS_C ----
            # w_sb[p, c, :] = weights row (c*128 + p); kg[j][p, c] lands in
            # exactly the per-chunk per-partition layout the ACT ops need.
            w_sb = kp.tile([128, 256], f32, tag="wsb")
            nc.sync.dma_start(
                w_sb[:].rearrange("p (c g) -> p c g", g=16),
                w_h.ap().rearrange("(c p) g -> p c g", p=128),
            )
            e_sb = kp.tile([128, 256], f32, tag="esb")
            nc.scalar.activation(e_sb[:], w_sb[:], AF.Exp)
            s_sb = kp.tile([128, NCHUNK], f32, tag="ssb")
            nc.vector.tensor_reduce(
                s_sb[:],
                e_sb[:].rearrange("p (c g) -> p c g", g=16),
                mybir.AxisListType.X,
                ALU.add,
            )
            r_sb = kp.tile([128, NCHUNK], f32, tag="rsb")
            nc.vector.reciprocal(r_sb[:], s_sb[:])
            gm_sb = kp.tile([128, 4, 256], f32, tag="gmsb")
            nc.sync.dma_start(
                gm_sb[:], gm_h.ap().rearrange("j p g -> p j g")
            )
            for j in range(4):
                t1 = kp.tile([128, 256], f32, tag="t1", name=f"t1_{j}")
                nc.vector.tensor_mul(t1[:], e_sb[:], gm_sb[:, j, :])
                kraw = kp.tile([128, NCHUNK], f32, tag="kraw", name=f"kraw{j}")
                nc.vector.tensor_reduce(
                    kraw[:],
                    t1[:].rearrange("p (c g) -> p c g", g=16),
                    mybir.AxisListType.X,
                    ALU.add,
                )
                nc.vector.tensor_mul(kg[j][:], kraw[:], r_sb[:])

            # ---- gather + multilinear + store (all fp16, neuron-major) ----
            for t in range(NCHUNK):
                AB = pab.tile([128, 2, BATCH], f16, tag="AB")
                nc.gpsimd.dma_gather(
                    out_ap=AB[:],
                    in_ap=xT_h.ap()[:, :],
                    idxs_ap=iab_sb[:, t * 16 : (t + 1) * 16],
                    num_idxs=IDX_PER_OP,
                    num_idxs_reg=IDX_PER_OP,
                    elem_size=BATCH,
                )
                u = puw.tile([128, BATCH], f16, tag="u")
                w = puw.tile([128, BATCH], f16, tag="w")
                # u split: DVE tensor_scalar on an odd-length slice (2x_1P
                # mode, single read port — leaves the DVE/GpSimd shared SBUF
                # port free for gather descriptor emission), ACT does the rest.
                nc.vector.tensor_scalar(
                    u[:, 0:2047],
                    AB[:, 0, 0:2047],
                    kg[1][:, t : t + 1],
                    kg[0][:, t : t + 1],
                    ALU.mult,
                    ALU.add,
                )
                nc.scalar.activation(
                    u[:, 2047:BATCH],
                    AB[:, 0, 2047:BATCH],
                    AF.Identity,
                    bias=kg[0][:, t : t + 1],
                    scale=kg[1][:, t : t + 1],
                )
                nc.scalar.activation(
                    w[:],
                    AB[:, 0, :],
                    AF.Identity,
                    bias=kg[2][:, t : t + 1],
                    scale=kg[3][:, t : t + 1],
                )
                nc.vector.tensor_mul(w[:], w[:], AB[:, 1, :])
                if t % 2 == 0:
                    osb = po.tile([128, 2, BATCH], f16, tag="osb")
                nc.vector.tensor_add(osb[:, t % 2, :], u[:], w[:])
                if t % 2 == 1:
                    # store two chunks per DMA op (2MB) to amortize per-op cost
                    nc.sync.dma_start(
                        out_h.ap()[(t - 1) * 128 : (t + 1) * 128, :].rearrange(
                            "(c p) n -> p c n", p=128
                        ),
                        osb[:],
                    )

    nc.compile()
    return nc


def _host_inputs(x, weights, a_idx, b_idx):
    x = np.asarray(x, dtype=np.float32)
    weights = np.asarray(weights, dtype=np.float32)
    a_idx = np.asarray(a_idx)
    b_idx = np.asarray(b_idx)
    xT = np.ascontiguousarray((x.T - np.float32(_H)).astype(np.float16))
    gm = np.ascontiguousarray(
        np.broadcast_to(
            np.tile(_GATE_COEFFS_C.T, (1, 16))[:, None, :], (4, 128, 256)
        )
    ).astype(np.float32)
    in_maps = []
    for c in range(NCORES):
        sl = slice(c * OC, (c + 1) * OC)
        in_maps.append(
            {
                "xT": xT,
                "w16c": np.ascontiguousarray(weights[sl]),
                "iab": _wrap_idx(a_idx[sl], b_idx[sl]),
                "gm": gm,
            }
        )
    return in_maps


def kernel(x, weights, a_idx, b_idx):
    from concourse.bass_utils import run_bass_kernel_spmd

    if "nc" not in _PROGRAM_CACHE:
        _PROGRAM_CACHE["nc"] = _build_program()
    nc = _PROGRAM_CACHE["nc"]

    in_maps = _host_inputs(x, weights, a_idx, b_idx)
    res = run_bass_kernel_spmd(nc, in_maps, list(range(NCORES)))
    outT = np.concatenate(
        [np.asarray(res.results[c]["out"]) for c in range(NCORES)], axis=0
    )
    return outT.T.astype(np.float32)


# revision 18
# speedup vs baseline: 1.1637x; 1.1637x over previous
"""Trainium2 Bass kernel for the LogicLayer (gnn_message_passing) problem.

out[n, y] = k0[y] + k1[y]*a + k2[y]*b + k3[y]*(a*b)
  with a = x[n, a_idx[y]], b = x[n, b_idx[y]],
  k = softmax(weights, -1) @ GATE_COEFFS          (per output neuron y)

Strategy (8 NeuronCores, sharded over out_dim — 2048 neurons/core, full
batch). The kernel is HBM-bandwidth bound, so all bulk traffic is 16-bit:
  * x is uploaded transposed and recentered in fp16: xT = fp16(x - 0.5),
    [16384, 4096] feature-major. The 0.5 shift halves the fp16
    quantization error (x' in [-0.5, 0.5)) and is folded into the gate
    coefficient matrix on the host (out = C0 + C1 a' + C2 b' + C3 a'b').
  * Per-core on-device softmax of the core's weight slice gives coefficient
    tiles kg[j][q, t] = C_j(y = t*128 + q) directly.
  * Per 128-output chunk t: ONE dma_gather with 256 indices (the chunk's
    a-rows then b-rows, 8KB/descriptor, 2MB/op) lands AB [128, 2, 4096]
    in SBUF. u = C1*A + C0 is split between DVE tensor_scalar (odd-length
    slice -> 2x_1P single-port mode) and ACT; ACT computes w = C3*A + C2
    (free affine); DVE computes w *= B and osb = u + w (fp16 tensor_tensor,
    2x mode); one contiguous 2MB DMA stores each pair of output row blocks.
  * Output is produced neuron-major ([out, batch] fp16); the host
    transposes/casts back to [batch, out] f32. Total HBM traffic per core:
    32MB gather + 16MB store (vs 64+32 for the f32 batch-major variant).
"""

import numpy as np

_GATE_COEFFS = np.array(
    [
        [0.0, 0.0, 0.0, 0.0],
        [0.0, 0.0, 0.0, 1.0],
        [0.0, 1.0, 0.0, -1.0],
        [0.0, 1.0, 0.0, 0.0],
        [0.0, 0.0, 1.0, -1.0],
        [0.0, 0.0, 1.0, 0.0],
        [0.0, 1.0, 1.0, -2.0],
        [0.0, 1.0, 1.0, -1.0],
        [1.0, -1.0, -1.0, 1.0],
        [1.0, -1.0, -1.0, 2.0],
        [1.0, 0.0, -1.0, 0.0],
        [1.0, 0.0, -1.0, 1.0],
        [1.0, -1.0, 0.0, 0.0],
        [1.0, -1.0, 0.0, 1.0],
        [1.0, 0.0, 0.0, -1.0],
        [1.0, 0.0, 0.0, 0.0],
    ],
    dtype=np.float32,
)

# x is stored recentered (x' = x - H); fold the shift into the coefficients:
# out = k0 + k1(a'+H) + k2(b'+H) + k3(a'+H)(b'+H) = C0 + C1 a' + C2 b' + C3 a'b'
_H = 0.5
_SHIFT_T = np.array(
    [
        [1.0, _H, _H, _H * _H],
        [0.0, 1.0, 0.0, _H],
        [0.0, 0.0, 1.0, _H],
        [0.0, 0.0, 0.0, 1.0],
    ],
    dtype=np.float32,
)
_GATE_COEFFS_C = _GATE_COEFFS @ _SHIFT_T.T  # [16, 4]: softmax(w) @ this = C

BATCH, IN_DIM, OUT_DIM = 4096, 16384, 16384
NCORES = 8
OC = OUT_DIM // NCORES   # 2048 outputs per core
NCHUNK = OC // 128       # 16 chunks of 128 outputs
IDX_PER_OP = 256         # one chunk's a+b rows in one dma_gather
NGOP = OC * 2 // IDX_PER_OP  # 16 gather ops per core

_PROGRAM_CACHE = {}


def _wrap_idx(a_slice: np.ndarray, b_slice: np.ndarray) -> np.ndarray:
    """dma_gather wrapped-int16 layout. Gather op o fetches 512 rows
    (a[2o]*128, b[2o]*128, a[2o+1]*128, b[2o+1]*128 — idx i lands in
    partition i%128, slot i//128); item i of op o lives at
    [i % 16, o*32 + i//16], replicated across the 8 16-partition groups."""
    idx = np.stack([a_slice.reshape(NCHUNK, 128), b_slice.reshape(NCHUNK, 128)], 1)
    idx = idx.reshape(NGOP, IDX_PER_OP).astype(np.int16)        # [o, i]
    w = idx.reshape(NGOP, IDX_PER_OP // 16, 16)                 # [o, s, p16]
    w = np.ascontiguousarray(w.transpose(2, 0, 1)).reshape(16, NGOP * 16)
    return np.ascontiguousarray(np.tile(w, (8, 1)))


def _build_program():
    import concourse.bass as bass  # noqa: F401
    import concourse.tile as tile
    from concourse import bacc, mybir

    f32 = mybir.dt.float32
    f16 = mybir.dt.float16
    i16 = mybir.dt.int16
    AF = mybir.ActivationFunctionType
    ALU = mybir.AluOpType

    nc = bacc.Bacc("TRN2", target_bir_lowering=False, debug=False)
    xT_h = nc.dram_tensor("xT", [IN_DIM, BATCH], f16, kind="ExternalInput")
    w_h = nc.dram_tensor("w16c", [OC, 16], f32, kind="ExternalInput")
    iab_h = nc.dram_tensor("iab", [128, NGOP * 16], i16, kind="ExternalInput")
    gm_h = nc.dram_tensor("gm", [4, 128, 256], f32, kind="ExternalInput")
    out_h = nc.dram_tensor("out", [OC, BATCH], f16, kind="ExternalOutput")

    with tile.TileContext(nc) as tc:
        from contextlib import ExitStack

        with ExitStack() as stack:
            cp = stack.enter_context(tc.tile_pool(name="const", bufs=1))
            # all pools coexist: no SBUF reuse between the coefficient calc
            # and the main loop, so the first gathers start immediately and
            # overlap the (serial, small) softmax chain.
            kp = stack.enter_context(tc.tile_pool(name="kcalc", bufs=1))
            pab = stack.enter_context(tc.tile_pool(name="pab", bufs=6))
            puw = stack.enter_context(tc.tile_pool(name="puw", bufs=2))
            po = stack.enter_context(tc.tile_pool(name="po", bufs=6))

            iab_sb = cp.tile([128, NGOP * 16], i16)
            nc.sync.dma_start(iab_sb[:], iab_h.ap()[:, :])
            kg = [
                cp.tile([128, NCHUNK], f32, tag=f"kg{j}", name=f"kg{j}")
                for j in range(4)
            ]

            # ---- coefficients: C = softmax(weights_slice) @ GATE_COEFFS_C ----
            # w_sb[p, c, :] = weights row (c*128 + p); kg[j][p, c] lands in
            # exactly the per-chunk per-partition layout the ACT ops need.
            w_sb = kp.tile([128, 256], f32, tag="wsb")
            nc.sync.dma_start(
                w_sb[:].rearrange("p (c g) -> p c g", g=16),
                w_h.ap().rearrange("(c p) g -> p c g", p=128),
            )
            e_sb = kp.tile([128, 256], f32, tag="esb")
            nc.scalar.activation(e_sb[:], w_sb[:], AF.Exp)
            s_sb = kp.tile([128, NCHUNK], f32, tag="ssb")
            nc.vector.tensor_reduce(
                s_sb[:],
                e_sb[:].rearrange("p (c g) -> p c g", g=16),
                mybir.AxisListType.X,
                ALU.add,
            )
            r_sb = kp.tile([128, NCHUNK], f32, tag="rsb")
            nc.vector.reciprocal(r_sb[:], s_sb[:])
            gm_sb = kp.tile([128, 4, 256], f32, tag="gmsb")
            nc.sync.dma_start(
                gm_sb[:], gm_h.ap().rearrange("j p g -> p j g")
            )
            for j in range(4):
                t1 = kp.tile([128, 256], f32, tag="t1", name=f"t1_{j}")
                nc.vector.tensor_mul(t1[:], e_sb[:], gm_sb[:, j, :])
                kraw = kp.tile([128, NCHUNK], f32, tag="kraw", name=f"kraw{j}")
                nc.vector.tensor_reduce(
                    kraw[:],
                    t1[:].rearrange("p (c g) -> p c g", g=16),
                    mybir.AxisListType.X,
                    ALU.add,
                )
                nc.vector.tensor_mul(kg[j][:], kraw[:], r_sb[:])

            # ---- gather + multilinear + store (all fp16, neuron-major) ----
            for t in range(NCHUNK):
                AB = pab.tile([128, 2, BATCH], f16, tag="AB")
                nc.gpsimd.dma_gather(
                    out_ap=AB[:],
                    in_ap=xT_h.ap()[:, :],
                    idxs_ap=iab_sb[:, t * 16 : (t + 1) * 16],
                    num_idxs=IDX_PER_OP,
                    num_idxs_reg=IDX_PER_OP,
                    elem_size=BATCH,
                )
                u = puw.tile([128, BATCH], f16, tag="u")
                w = puw.tile([128, BATCH], f16, tag="w")
                # u split: DVE tensor_scalar on an odd-length slice (2x_1P
                # mode, single read port — leaves the DVE/GpSimd shared SBUF
                # port free for gather descriptor emission), ACT does the rest.
                nc.vector.tensor_scalar(
                    u[:, 0:2047],
                    AB[:, 0, 0:2047],
                    kg[1][:, t : t + 1],
                    kg[0][:, t : t + 1],
                    ALU.mult,
                    ALU.add,
                )
                nc.scalar.activation(
                    u[:, 2047:BATCH],
                    AB[:, 0, 2047:BATCH],
                    AF.Identity,
                    bias=kg[0][:, t : t + 1],
                    scale=kg[1][:, t : t + 1],
                )
                nc.scalar.activation(
                    w[:],
                    AB[:, 0, :],
                    AF.Identity,
                    bias=kg[2][:, t : t + 1],
                    scale=kg[3][:, t : t + 1],
                )
                nc.vector.tensor_mul(w[:], w[:], AB[:, 1, :])
                # single-chunk stores with deep osb buffering: under full
                # gather contention a store's SDMA slices crawl (~20us wall
                # for 1MB), so several must be in flight or the adds block
                # on osb reuse and the whole pipeline convoys.
                osb = po.tile([128, BATCH], f16, tag="osb")
                nc.vector.tensor_add(osb[:], u[:], w[:])
                nc.sync.dma_start(
                    out_h.ap()[t * 128 : (t + 1) * 128, :], osb[:]
                )

    nc.compile()
    return nc


def _host_inputs(x, weights, a_idx, b_idx):
    x = np.asarray(x, dtype=np.float32)
    weights = np.asarray(weights, dtype=np.float32)
    a_idx = np.asarray(a_idx)
    b_idx = np.asarray(b_idx)
    xT = np.ascontiguousarray((x.T - np.float32(_H)).astype(np.float16))
    gm = np.ascontiguousarray(
        np.broadcast_to(
            np.tile(_GATE_COEFFS_C.T, (1, 16))[:, None, :], (4, 128, 256)
        )
    ).astype(np.float32)
    in_maps = []
    for c in range(NCORES):
        sl = slice(c * OC, (c + 1) * OC)
        in_maps.append(
            {
                "xT": xT,
                "w16c": np.ascontiguousarray(weights[sl]),
                "iab": _wrap_idx(a_idx[sl], b_idx[sl]),
                "gm": gm,
            }
        )
    return in_maps


def kernel(x, weights, a_idx, b_idx):
    from concourse.bass_utils import run_bass_kernel_spmd

    if "nc" not in _PROGRAM_CACHE:
        _PROGRAM_CACHE["nc"] = _build_program()
    nc = _PROGRAM_CACHE["nc"]

    in_maps = _host_inputs(x, weights, a_idx, b_idx)
    res = run_bass_kernel_spmd(nc, in_maps, list(range(NCORES)))
    outT = np.concatenate(
        [np.asarray(res.results[c]["out"]) for c in range(NCORES)], axis=0
    )
    return outT.T.astype(np.float32)


# revision 19
# speedup vs baseline: 1.1874x; 1.0203x over previous
"""Trainium2 Bass kernel for the LogicLayer (gnn_message_passing) problem.

out[n, y] = k0[y] + k1[y]*a + k2[y]*b + k3[y]*(a*b)
  with a = x[n, a_idx[y]], b = x[n, b_idx[y]],
  k = softmax(weights, -1) @ GATE_COEFFS          (per output neuron y)

Strategy (8 NeuronCores, sharded over out_dim — 2048 neurons/core, full
batch). The kernel is HBM-bandwidth bound, so all bulk traffic is 16-bit:
  * x is uploaded transposed and recentered in fp16: xT = fp16(x - 0.5),
    [16384, 4096] feature-major. The 0.5 shift halves the fp16
    quantization error (x' in [-0.5, 0.5)) and is folded into the gate
    coefficient matrix on the host (out = C0 + C1 a' + C2 b' + C3 a'b').
  * Per-core on-device softmax of the core's weight slice gives coefficient
    tiles kg[j][q, t] = C_j(y = t*128 + q) directly.
  * Per 128-output chunk t: ONE dma_gather with 256 indices (the chunk's
    a-rows then b-rows, 8KB/descriptor, 2MB/op) lands AB [128, 2, 4096]
    in SBUF. u = C1*A + C0 is split between DVE tensor_scalar (odd-length
    slice -> 2x_1P single-port mode) and ACT; ACT computes w = C3*A + C2
    (free affine); DVE computes w *= B and osb = u + w (fp16 tensor_tensor,
    2x mode); one contiguous 2MB DMA stores each pair of output row blocks.
  * Output is produced neuron-major ([out, batch] fp16); the host
    transposes/casts back to [batch, out] f32. Total HBM traffic per core:
    32MB gather + 16MB store (vs 64+32 for the f32 batch-major variant).
"""

import numpy as np

_GATE_COEFFS = np.array(
    [
        [0.0, 0.0, 0.0, 0.0],
        [0.0, 0.0, 0.0, 1.0],
        [0.0, 1.0, 0.0, -1.0],
        [0.0, 1.0, 0.0, 0.0],
        [0.0, 0.0, 1.0, -1.0],
        [0.0, 0.0, 1.0, 0.0],
        [0.0, 1.0, 1.0, -2.0],
        [0.0, 1.0, 1.0, -1.0],
        [1.0, -1.0, -1.0, 1.0],
        [1.0, -1.0, -1.0, 2.0],
        [1.0, 0.0, -1.0, 0.0],
        [1.0, 0.0, -1.0, 1.0],
        [1.0, -1.0, 0.0, 0.0],
        [1.0, -1.0, 0.0, 1.0],
        [1.0, 0.0, 0.0, -1.0],
        [1.0, 0.0, 0.0, 0.0],
    ],
    dtype=np.float32,
)

# x is stored recentered (x' = x - H); fold the shift into the coefficients:
# out = k0 + k1(a'+H) + k2(b'+H) + k3(a'+H)(b'+H) = C0 + C1 a' + C2 b' + C3 a'b'
_H = 0.5
_SHIFT_T = np.array(
    [
        [1.0, _H, _H, _H * _H],
        [0.0, 1.0, 0.0, _H],
        [0.0, 0.0, 1.0, _H],
        [0.0, 0.0, 0.0, 1.0],
    ],
    dtype=np.float32,
)
_GATE_COEFFS_C = _GATE_COEFFS @ _SHIFT_T.T  # [16, 4]: softmax(w) @ this = C

BATCH, IN_DIM, OUT_DIM = 4096, 16384, 16384
NCORES = 8
OC = OUT_DIM // NCORES   # 2048 outputs per core
NCHUNK = OC // 128       # 16 chunks of 128 outputs
IDX_PER_OP = 256         # one chunk's a+b rows in one dma_gather
NGOP = OC * 2 // IDX_PER_OP  # 16 gather ops per core

_PROGRAM_CACHE = {}


def _wrap_idx(a_slice: np.ndarray, b_slice: np.ndarray) -> np.ndarray:
    """dma_gather wrapped-int16 layout. Gather op o fetches 512 rows
    (a[2o]*128, b[2o]*128, a[2o+1]*128, b[2o+1]*128 — idx i lands in
    partition i%128, slot i//128); item i of op o lives at
    [i % 16, o*32 + i//16], replicated across the 8 16-partition groups."""
    idx = np.stack([a_slice.reshape(NCHUNK, 128), b_slice.reshape(NCHUNK, 128)], 1)
    idx = idx.reshape(NGOP, IDX_PER_OP).astype(np.int16)        # [o, i]
    w = idx.reshape(NGOP, IDX_PER_OP // 16, 16)                 # [o, s, p16]
    w = np.ascontiguousarray(w.transpose(2, 0, 1)).reshape(16, NGOP * 16)
    return np.ascontiguousarray(np.tile(w, (8, 1)))


def _build_program():
    import concourse.bass as bass  # noqa: F401
    import concourse.tile as tile
    from concourse import bacc, mybir

    f32 = mybir.dt.float32
    f16 = mybir.dt.float16
    i16 = mybir.dt.int16
    AF = mybir.ActivationFunctionType
    ALU = mybir.AluOpType

    nc = bacc.Bacc("TRN2", target_bir_lowering=False, debug=False)
    xT_h = nc.dram_tensor("xT", [IN_DIM, BATCH], f16, kind="ExternalInput")
    w_h = nc.dram_tensor("w16c", [OC, 16], f32, kind="ExternalInput")
    iab_h = nc.dram_tensor("iab", [128, NGOP * 16], i16, kind="ExternalInput")
    gm_h = nc.dram_tensor("gm", [4, 128, 256], f32, kind="ExternalInput")
    out_h = nc.dram_tensor("out", [OC, BATCH], f16, kind="ExternalOutput")

    with tile.TileContext(nc) as tc:
        from contextlib import ExitStack

        with ExitStack() as stack:
            cp = stack.enter_context(tc.tile_pool(name="const", bufs=1))
            # all pools coexist: no SBUF reuse between the coefficient calc
            # and the main loop, so the first gathers start immediately and
            # overlap the (serial, small) softmax chain.
            kp = stack.enter_context(tc.tile_pool(name="kcalc", bufs=1))
            pab = stack.enter_context(tc.tile_pool(name="pab", bufs=5))
            puw = stack.enter_context(tc.tile_pool(name="puw", bufs=2))
            po = stack.enter_context(tc.tile_pool(name="po", bufs=4))

            iab_sb = cp.tile([128, NGOP * 16], i16)
            nc.sync.dma_start(iab_sb[:], iab_h.ap()[:, :])
            kg = [
                cp.tile([128, NCHUNK], f32, tag=f"kg{j}", name=f"kg{j}")
                for j in range(4)
            ]

            # ---- coefficients: C = softmax(weights_slice) @ GATE_COEFFS_C ----
            # w_sb[p, c, :] = weights row (c*128 + p); kg[j][p, c] lands in
            # exactly the per-chunk per-partition layout the ACT ops need.
            w_sb = kp.tile([128, 256], f32, tag="wsb")
            nc.sync.dma_start(
                w_sb[:].rearrange("p (c g) -> p c g", g=16),
                w_h.ap().rearrange("(c p) g -> p c g", p=128),
            )
            e_sb = kp.tile([128, 256], f32, tag="esb")
            nc.scalar.activation(e_sb[:], w_sb[:], AF.Exp)
            s_sb = kp.tile([128, NCHUNK], f32, tag="ssb")
            nc.vector.tensor_reduce(
                s_sb[:],
                e_sb[:].rearrange("p (c g) -> p c g", g=16),
                mybir.AxisListType.X,
                ALU.add,
            )
            r_sb = kp.tile([128, NCHUNK], f32, tag="rsb")
            nc.vector.reciprocal(r_sb[:], s_sb[:])
            gm_sb = kp.tile([128, 4, 256], f32, tag="gmsb")
            nc.sync.dma_start(
                gm_sb[:], gm_h.ap().rearrange("j p g -> p j g")
            )
            for j in range(4):
                t1 = kp.tile([128, 256], f32, tag="t1", name=f"t1_{j}")
                nc.vector.tensor_mul(t1[:], e_sb[:], gm_sb[:, j, :])
                kraw = kp.tile([128, NCHUNK], f32, tag="kraw", name=f"kraw{j}")
                nc.vector.tensor_reduce(
                    kraw[:],
                    t1[:].rearrange("p (c g) -> p c g", g=16),
                    mybir.AxisListType.X,
                    ALU.add,
                )
                nc.vector.tensor_mul(kg[j][:], kraw[:], r_sb[:])

            # ---- gather + multilinear + store (all fp16, neuron-major) ----
            for t in range(NCHUNK):
                AB = pab.tile([128, 2, BATCH], f16, tag="AB")
                nc.gpsimd.dma_gather(
                    out_ap=AB[:],
                    in_ap=xT_h.ap()[:, :],
                    idxs_ap=iab_sb[:, t * 16 : (t + 1) * 16],
                    num_idxs=IDX_PER_OP,
                    num_idxs_reg=IDX_PER_OP,
                    elem_size=BATCH,
                )
                u = puw.tile([128, BATCH], f16, tag="u")
                w = puw.tile([128, BATCH], f16, tag="w")
                # u split: DVE tensor_scalar on an odd-length slice (2x_1P
                # mode, single read port — leaves the DVE/GpSimd shared SBUF
                # port free for gather descriptor emission), ACT does the rest.
                nc.vector.tensor_scalar(
                    u[:, 0:2047],
                    AB[:, 0, 0:2047],
                    kg[1][:, t : t + 1],
                    kg[0][:, t : t + 1],
                    ALU.mult,
                    ALU.add,
                )
                nc.scalar.activation(
                    u[:, 2047:BATCH],
                    AB[:, 0, 2047:BATCH],
                    AF.Identity,
                    bias=kg[0][:, t : t + 1],
                    scale=kg[1][:, t : t + 1],
                )
                nc.scalar.activation(
                    w[:],
                    AB[:, 0, :],
                    AF.Identity,
                    bias=kg[2][:, t : t + 1],
                    scale=kg[3][:, t : t + 1],
                )
                nc.vector.tensor_mul(w[:], w[:], AB[:, 1, :])
                # 2-chunk stores + deep osb slack: under full gather
                # contention a store's SDMA slices crawl, so several must be
                # in flight or the adds block on osb reuse; fewer store ops
                # also keeps the 8 round-robin DMA-completion-sem lanes from
                # pairing a gather wait with a slow store's increments.
                if t % 2 == 0:
                    osb = po.tile([128, 2, BATCH], f16, tag="osb")
                nc.vector.tensor_add(osb[:, t % 2, :], u[:], w[:])
                if t % 2 == 1:
                    nc.sync.dma_start(
                        out_h.ap()[(t - 1) * 128 : (t + 1) * 128, :].rearrange(
                            "(c p) n -> p c n", p=128
                        ),
                        osb[:],
                    )

    nc.compile()
    return nc


def _host_inputs(x, weights, a_idx, b_idx):
    x = np.asarray(x, dtype=np.float32)
    weights = np.asarray(weights, dtype=np.float32)
    a_idx = np.asarray(a_idx)
    b_idx = np.asarray(b_idx)
    xT = np.ascontiguousarray((x.T - np.float32(_H)).astype(np.float16))
    gm = np.ascontiguousarray(
        np.broadcast_to(
            np.tile(_GATE_COEFFS_C.T, (1, 16))[:, None, :], (4, 128, 256)
        )
    ).astype(np.float32)
    in_maps = []
    for c in range(NCORES):
        sl = slice(c * OC, (c + 1) * OC)
        in_maps.append(
            {
                "xT": xT,
                "w16c": np.ascontiguousarray(weights[sl]),
                "iab": _wrap_idx(a_idx[sl], b_idx[sl]),
                "gm": gm,
            }
        )
    return in_maps


def kernel(x, weights, a_idx, b_idx):
    from concourse.bass_utils import run_bass_kernel_spmd

    if "nc" not in _PROGRAM_CACHE:
        _PROGRAM_CACHE["nc"] = _build_program()
    nc = _PROGRAM_CACHE["nc"]

    in_maps = _host_inputs(x, weights, a_idx, b_idx)
    res = run_bass_kernel_spmd(nc, in_maps, list(range(NCORES)))
    outT = np.concatenate(
        [np.asarray(res.results[c]["out"]) for c in range(NCORES)], axis=0
    )
    return outT.T.astype(np.float32)
